# revision 8
# baseline (speedup 1.0000x reference)
"""APPNP (MLP + 10-step personalized-pagerank propagation) on 8 trn2 NeuronCores.

Strategy:
- Nodes are dst-sharded across 8 cores (12500 each).
- MLP (x @ W1 -> relu -> @ W2) runs on the tensor engine per core over the
  core's node shard, with x pre-transposed on host (contraction dim on
  partitions) and b1 folded in as an extra ones-row of x.
- Propagation uses the factorized GCN norm: A_hat h = dinv * ((A+I) (dinv*h)):
  per step each core computes g = dinv*h on its shard, AllGathers g into a
  full table in DRAM (packed as 256B granules of 4 node-rows), then gathers
  the granule containing g[src] for each in-edge of its shard with
  gpsimd.dma_gather (vectorized Q7 descriptor generation, ~0.4ns/desc vs
  ~26ns/desc for the generic indirect-DMA path), selects the right row of
  each granule with a one-hot mask multiply, and reduces slots per dst with
  one XY vector reduction per chunk; finally adds self-loop + alpha*h0 terms.
- Slot layout: per core, dsts sorted by in-degree desc; sorted position
  s <-> (block b = s//128, lane = s%128). Block b's slot-column count = max
  degree in block (degree-sorted => tiny padding). Blocks are greedily packed
  into chunks of <= 64 slot columns = 8192 gather indices per dma_gather
  (larger single gathers crash the exec unit). Pad slots gather a zeroed
  granule and carry an all-zero mask.
"""
import numpy as np

_LAST_NC = None
_LAST_IN_MAPS = None
_LAST_PLAN = None

K = 10
ALPHA = 0.1
N_NODES = 100000
N_CORES = 8
NS = N_NODES // N_CORES          # 12500 dsts per core
NB = 98                           # ceil(12544/128) blocks (12544 = 128*98)
NRANK = 128 * NB                  # 12544 padded ranks per core
IN_CH, HID_CH, OUT_CH = 500, 64, 16
KIN = 512                         # padded in_ch (500 feats + 1 bias + pad)
P = 128

GR = 4                            # fp32 rows per 256B granule
SHARD_GRAN = NRANK // GR + 1      # 3137 granules/shard (+1 zero granule)
SHARD_ROWS = SHARD_GRAN * GR      # 12548
NGRAN = SHARD_GRAN * N_CORES      # 25096 total granules (< 2**15 for int16)
ZERO_GRAN = NRANK // GR           # shard-0-relative zero granule id (3136)
ELEM = GR * OUT_CH                # 64 fp32 per granule
CHUNK_COLS = 64                   # slot columns per dma_gather (8192 idxs)


def _plan_chunks(d_b):
    """Greedy-pack whole blocks into chunks of <= CHUNK_COLS slot columns.

    Returns (chunks, col_off): chunks = [(b0, nb, c0, cols)], col_off[b] =
    first global slot column of block b.
    """
    col_off = np.zeros(NB + 1, dtype=np.int64)
    col_off[1:] = np.cumsum(d_b)
    chunks = []
    b = 0
    while b < NB:
        nb = 1
        while (b + nb < NB
               and col_off[b + nb + 1] - col_off[b] <= CHUNK_COLS):
            nb += 1
        chunks.append((b, nb, int(col_off[b]),
                       int(col_off[b + nb] - col_off[b])))
        b += nb
    return chunks, col_off


def _build_host_data(x, edge_index, W1, b1, W2, b2):
    x = np.asarray(x, dtype=np.float32)
    ei = np.asarray(edge_index)
    src = ei[0].astype(np.int64)
    dst = ei[1].astype(np.int64)

    deg = np.bincount(dst, minlength=N_NODES).astype(np.float32) + 1.0
    dinv = 1.0 / np.sqrt(deg)

    # per-core degree sort of the core's dst shard; global row map for g table
    row_of_node = np.empty(N_NODES, dtype=np.int64)
    perm_per_core = []          # natural ids in sorted order per core
    for c in range(N_CORES):
        ids = np.arange(c * NS, (c + 1) * NS)
        order = np.argsort(-deg[ids], kind="stable")
        ids_sorted = ids[order]
        perm_per_core.append(ids_sorted)
        s = np.arange(NS)
        row_of_node[ids_sorted] = c * SHARD_ROWS + (s % P) * NB + s // P

    per_core = []
    dst_core = dst // NS
    for c in range(N_CORES):
        m = dst_core == c
        src_c = src[m]
        dst_c = dst[m]
        ids_sorted = perm_per_core[c]
        pos_of = np.empty(NS, dtype=np.int64)
        pos_of[ids_sorted - c * NS] = np.arange(NS)
        pos = pos_of[dst_c - c * NS]
        degs = deg[ids_sorted].astype(np.int64) - 1   # in-edges only
        d_b = np.zeros(NB, dtype=np.int64)
        for b in range(NB):
            seg = degs[b * P:(b + 1) * P]
            d_b[b] = seg.max() if len(seg) else 0

        order2 = np.lexsort((src_c, pos))   # group edges by dst pos
        pos_s = pos[order2]
        src_s = src_c[order2]
        counts = np.bincount(pos_s, minlength=NS)
        rank = np.arange(len(pos_s)) - np.repeat(
            np.concatenate(([0], np.cumsum(counts)))[:-1], counts)
        per_core.append(dict(d_b=d_b, pos_s=pos_s, rank=rank,
                             src_rows=row_of_node[src_s],
                             ids_sorted=ids_sorted))

    # MLP host prep per core: xT [128, 4, NRANK] fp32, column order = sorted pos
    W1p = np.zeros((KIN, HID_CH), dtype=np.float32)
    W1p[:IN_CH] = np.asarray(W1, dtype=np.float32)
    W1p[IN_CH] = np.asarray(b1, dtype=np.float32)
    W1p_t = W1p.reshape(4, P, HID_CH).transpose(1, 0, 2).copy()  # [128,4,64]
    for c in range(N_CORES):
        ids_sorted = per_core[c]["ids_sorted"]
        xp = np.zeros((KIN, NRANK), dtype=np.float32)
        xp[:IN_CH, :NS] = x[ids_sorted].T
        xp[IN_CH, :NS] = 1.0
        per_core[c]["xT"] = xp.reshape(4, P, NRANK).transpose(1, 0, 2).copy()
        dv = np.zeros((P, NB), dtype=np.float32)
        s = np.arange(NS)
        dv[s % P, s // P] = dinv[ids_sorted]
        per_core[c]["dinv"] = dv
    return per_core, W1p_t, np.asarray(W2, np.float32), np.asarray(b2, np.float32)


def _build_bass(chunks, col_off, d_b, T_g, n_queues=4, k_steps=K,
                slot_bufs=3):
    import concourse.bacc as bacc
    import concourse.mybir as mybir
    import concourse.tile as tile

    nc = bacc.Bacc(None, num_devices=N_CORES, num_swdge_queues=n_queues,
                   dynamic_dma_scratch_size=65536)
    xT = nc.dram_tensor("xT", [P, 4, NRANK], mybir.dt.float32, kind="ExternalInput")
    W1p = nc.dram_tensor("W1p", [P, 4, HID_CH], mybir.dt.float32, kind="ExternalInput")
    W2 = nc.dram_tensor("W2", [HID_CH, OUT_CH], mybir.dt.float32, kind="ExternalInput")
    b2 = nc.dram_tensor("b2", [OUT_CH, 1], mybir.dt.float32, kind="ExternalInput")
    dinv_in = nc.dram_tensor("dinv", [P, NB], mybir.dt.float32, kind="ExternalInput")
    idx_in = nc.dram_tensor("idx", [P, T_g * 8], mybir.dt.int16, kind="ExternalInput")
    mask_in = nc.dram_tensor("mask", [P, T_g * GR], mybir.dt.float32, kind="ExternalInput")
    h_out = nc.dram_tensor("h_out", [P, NB * OUT_CH], mybir.dt.float32, kind="ExternalOutput")

    gshard = nc.dram_tensor("gshard", [SHARD_GRAN, ELEM], mybir.dt.float32)
    Gtab = nc.dram_tensor("Gtab", [NGRAN, ELEM], mybir.dt.float32)

    dt = mybir.dt.float32
    with tile.TileContext(nc) as tc:
        with tc.tile_pool(name="persist", bufs=1) as pers, \
             tc.tile_pool(name="ps", bufs=2, space="PSUM") as pp, \
             tc.tile_pool(name="pst", bufs=2, space="PSUM") as ppt:

            # persistent tiles
            idx_t = pers.tile([P, T_g * 8], mybir.dt.int16)
            nc.sync.dma_start(idx_t[:], idx_in[:])
            dinv_t = pers.tile([P, NB], dt)
            nc.sync.dma_start(dinv_t[:], dinv_in[:])
            w1_t = pers.tile([P, 4, HID_CH], dt)
            nc.sync.dma_start(w1_t[:], W1p[:])
            w2_t = pers.tile([HID_CH, OUT_CH], dt)
            nc.sync.dma_start(w2_t[:], W2[:])
            b2_t = pers.tile([OUT_CH, 1], dt)
            nc.sync.dma_start(b2_t[:], b2[:])
            ident = pers.tile([P, P], dt)
            from concourse.masks import make_identity
            make_identity(nc, ident[:])

            h0s_t = pers.tile([P, NB, OUT_CH], dt)   # alpha * h0
            h_t = pers.tile([P, NB, OUT_CH], dt)     # current h
            g_t = pers.tile([P, NB, OUT_CH], dt)     # dinv * h
            agg_t = pers.tile([P, NB, OUT_CH], dt)
            zgran = pers.tile([1, ELEM], dt)
            nc.gpsimd.memset(zgran[:], 0.0)
            nc.gpsimd.dma_start(gshard.ap()[ZERO_GRAN:ZERO_GRAN + 1, :], zgran[:])

            # ---- MLP ----
            mlp_scope = tc.tile_pool(name="mlp", bufs=3)
            mpool = mlp_scope.__enter__()
            tiles = [(t * KIN, KIN) for t in range(NRANK // KIN)]
            rem = NRANK - (NRANK // KIN) * KIN
            if rem:
                tiles.append(((NRANK // KIN) * KIN, rem))
            for (c0, w) in tiles:
                xt = mpool.tile([P, 4, KIN], dt, tag="xt")
                nc.sync.dma_start(xt[:, :, :w], xT[:, :, c0:c0 + w])
                ps1 = pp.tile([HID_CH, KIN], dt, tag="ps1")
                for k in range(4):
                    nc.tensor.matmul(ps1[:, :w], w1_t[:, k, :], xt[:, k, :w],
                                     start=(k == 0), stop=(k == 3))
                h1 = mpool.tile([HID_CH, KIN], dt, tag="h1")
                nc.vector.tensor_scalar_max(h1[:, :w], ps1[:, :w], 0.0)
                ps2 = pp.tile([OUT_CH, KIN], dt, tag="ps2")
                nc.tensor.matmul(ps2[:, :w], w2_t[:], h1[:, :w],
                                 start=True, stop=True)
                hT = mpool.tile([OUT_CH, KIN], dt, tag="hT")
                nc.vector.tensor_tensor(hT[:, :w], ps2[:, :w],
                                        b2_t[:].to_broadcast([OUT_CH, w]),
                                        op=mybir.AluOpType.add)
                for j in range(w // P):
                    b = (c0 + j * P) // P
                    pst = ppt.tile([P, OUT_CH], dt, tag="pst")
                    nc.tensor.transpose(pst[:], hT[:, j * P:(j + 1) * P],
                                        ident[:OUT_CH, :OUT_CH])
                    nc.vector.tensor_copy(h0s_t[:, b, :], pst[:])
            # h = h0 ; h0s = alpha*h0
            nc.vector.tensor_copy(h_t[:], h0s_t[:])
            nc.vector.tensor_scalar_mul(h0s_t[:], h0s_t[:], ALPHA)
            mlp_scope.__exit__(None, None, None)
            slot_scope = tc.tile_pool(name="slot", bufs=slot_bufs)
            spool = slot_scope.__enter__()
            mask_scope = tc.tile_pool(name="maskp", bufs=slot_bufs)
            kpool = mask_scope.__enter__()

            # ---- propagation steps ----
            def step_body(_i):
                # g = dinv * h
                nc.vector.tensor_tensor(
                    g_t[:], h_t[:],
                    dinv_t[:].rearrange("p (b o) -> p b o", o=1).to_broadcast([P, NB, OUT_CH]),
                    op=mybir.AluOpType.mult)
                nc.sync.dma_start(gshard.ap()[:ZERO_GRAN, :], g_t[:])
                nc.gpsimd.collective_compute(
                    "AllGather", mybir.AluOpType.bypass,
                    replica_groups=[list(range(N_CORES))],
                    ins=[gshard.ap()[:, :]],
                    outs=[Gtab.ap()[:, :]],
                )
                for ci, (b0, nb, c0, cols) in enumerate(chunks):
                    st = spool.tile([P, CHUNK_COLS, ELEM], dt, tag="slot")
                    nc.gpsimd.dma_gather(
                        st[:, :cols, :], Gtab[:],
                        idx_t[:, c0 * 8:(c0 + cols) * 8],
                        cols * P, cols * P, ELEM,
                        single_packet=False, queue_num=ci % n_queues)
                    mk = kpool.tile([P, CHUNK_COLS, GR], dt, tag="mask")
                    nc.sync.dma_start(mk[:, :cols, :],
                                      mask_in[:, c0 * GR:(c0 + cols) * GR])
                    stv = st[:, :cols, :].rearrange("p s (g c) -> p s g c", g=GR)
                    nc.vector.tensor_tensor(
                        stv, stv,
                        mk[:, :cols, :].rearrange("p s (g o) -> p s g o", o=1)
                        .to_broadcast([P, cols, GR, OUT_CH]),
                        op=mybir.AluOpType.mult)
                    # reduce slots+subrows per dst block
                    for b in range(b0, b0 + nb):
                        off = int(col_off[b]) - c0
                        db = int(d_b[b])
                        nc.vector.reduce_sum(
                            agg_t[:, b, :],
                            st[:, off:off + db, :].rearrange(
                                "p s (g c) -> p c s g", g=GR),
                            axis=mybir.AxisListType.XY)
                # h = 0.9 * dinv * (agg + g) + alpha*h0
                nc.vector.tensor_add(agg_t[:], agg_t[:], g_t[:])
                nc.vector.tensor_tensor(
                    agg_t[:], agg_t[:],
                    dinv_t[:].rearrange("p (b o) -> p b o", o=1).to_broadcast([P, NB, OUT_CH]),
                    op=mybir.AluOpType.mult)
                nc.vector.tensor_scalar_mul(agg_t[:], agg_t[:], 1.0 - ALPHA)
                nc.vector.tensor_add(h_t[:], agg_t[:], h0s_t[:])

            for _step in range(k_steps):
                step_body(_step)

            nc.sync.dma_start(h_out[:], h_t[:])
            mask_scope.__exit__(None, None, None)
            slot_scope.__exit__(None, None, None)
    nc.compile()
    return nc


def _host_tables(per_core):
    """Shared slot schedule + per-core idx16/mask arrays."""
    d_b = np.max(np.stack([pc["d_b"] for pc in per_core]), axis=0)
    chunks, col_off = _plan_chunks(d_b)
    T_g = int(d_b.sum())

    per_core_tabs = []
    for pc in per_core:
        lane_s = pc["pos_s"] % P
        blk_s = pc["pos_s"] // P
        cols = col_off[blk_s] + pc["rank"]
        n = cols * P + lane_s                      # flat gather position
        idx_flat = np.full(T_g * P, ZERO_GRAN, dtype=np.int16)
        idx_flat[n] = (pc["src_rows"] // GR).astype(np.int16)
        idx_tile = np.tile(idx_flat.reshape(T_g * 8, 16).T, (8, 1)).copy()
        mask = np.zeros((P, T_g, GR), dtype=np.float32)
        mask[lane_s, cols, pc["src_rows"] % GR] = 1.0
        per_core_tabs.append((idx_tile, mask.reshape(P, T_g * GR)))
    return d_b, chunks, col_off, T_g, per_core_tabs


def _make_in_maps(per_core, per_core_tabs, W1p_t, W2a, b2a):
    in_maps = []
    for pc, (idx_tile, mask) in zip(per_core, per_core_tabs):
        in_maps.append({
            "xT": pc["xT"],
            "W1p": W1p_t,
            "W2": W2a,
            "b2": b2a.reshape(OUT_CH, 1),
            "dinv": pc["dinv"],
            "idx": idx_tile,
            "mask": mask,
        })
    return in_maps


def kernel(x, edge_index, W1, b1, W2, b2):
    per_core, W1p_t, W2a, b2a = _build_host_data(x, edge_index, W1, b1, W2, b2)
    d_b, chunks, col_off, T_g, per_core_tabs = _host_tables(per_core)
    in_maps = _make_in_maps(per_core, per_core_tabs, W1p_t, W2a, b2a)

    nc = _build_bass(chunks, col_off, d_b, T_g)
    global _LAST_NC, _LAST_IN_MAPS, _LAST_PLAN
    _LAST_NC, _LAST_IN_MAPS = nc, in_maps
    _LAST_PLAN = (chunks, col_off, d_b, T_g)
    from concourse import bass_utils
    res = bass_utils.run_bass_kernel_spmd(nc, in_maps, core_ids=list(range(N_CORES)))

    out = np.zeros((N_NODES, OUT_CH), dtype=np.float32)
    for c in range(N_CORES):
        hc = res.results[c]["h_out"].reshape(P, NB, OUT_CH)
        ids_sorted = per_core[c]["ids_sorted"]
        s = np.arange(NS)
        out[ids_sorted] = hc[s % P, s // P, :]
    return out
